# revision 8
# baseline (speedup 1.0000x reference)
"""AM-Softmax loss (margin=0.3, scale=30, label smoothing 0.1) on 8 TRN2 cores.

Vocab-parallel: classifier weight (C,d) sharded along C across 8 cores.

Main restructure vs the 314us baseline: the 25.6M-element exp+sum per core
(the ACT-engine bottleneck at ~154 G elem/s) is split between ACT (native
Exp with free accumulator) and DVE (Schraudolph bit-trick exp: one
fused mul+add with f32->i16 convert, then a bf16-bitcast tensor_reduce at
the 16-bit 2x/4x DVE rate).  Feature normalization is folded into the
per-sample (per-partition) activation scale / convert scale, so f is never
normalized explicitly; weight normalization is folded into the f32->bf16
cast on the Pool engine (tensor_scalar by 1/|w| per class row).

  loss_i = lse_i - 0.9*(l_i - 9) - (0.1/C)*(T_i - 9)
  lse_i  = 30 + ln(Z_i + (e^-9 - 1) * e^(l_i - 30))
  Z_i    = sum_c exp(30*cos_ic - 30)     (ACT exact / DVE bit-trick chunks)
  T_i    = 30 * invF_i * (f_i . s),  s = sum_c w_hat_c  (per-batch DVE
           X-reduce of wT accumulated into one [P,1] column)
  l_i    = 30 * f_hat_i . w_hat_{y_i}    (indirect-DMA gather on owner)

The w pipeline is software-pipelined with the main loop (batch k+2 DMA /
batch k+1 normalize+transpose / chunk k matmul+exp).  One AllReduce(add)
on packed [Z|T|L] at the end replaces the baseline's two AllGathers plus
local rank-reduce; the final log is a DVE bitcast-ln (no ACT table load).
"""

import math

import numpy as np

import concourse.bass as bass
import concourse.bacc as bacc
import concourse.mybir as mybir
from concourse import tile

P = 128
B, D, C = 2048, 128, 100000
NCORES = 8
CS = C // NCORES
S, MARG, EPS = 30.0, 0.3, 0.1

f32 = mybir.dt.float32
bf16 = mybir.dt.bfloat16
i16 = mybir.dt.int16
i32 = mybir.dt.int32
FT = mybir.ActivationFunctionType
OP = mybir.AluOpType

MAGIC = 0x5F3759DF
K_SCHR = 128.0 / math.log(2.0)     # bf16 exponent scale
C_CORR = 7.5                       # Schraudolph bias calibration
LN2 = math.log(2.0)

# ---- tunables --------------------------------------------------------------
CHUNK = 2048          # psum chunk columns (4 banks; x2 bufs = full PSUM)
PE_TP = 4             # weight batches transposed on PE (rest via DMA xbar)
ND_BY_CC = [5, 5, 5, 6, 7, 7, 7]   # DVE-assigned chunks per 16-chunk group


def dve_positions(nd):
    """Spread nd DVE-chunk positions evenly over 16 b-slots."""
    return {int(round(i * 16 / nd)) % 16 for i in range(nd)} if nd else set()


def build_graph(num_cores=NCORES, b_sz=B, cs=CS, chunk=CHUNK, wbatch=16):
    nb = b_sz // P                      # 16 B tiles
    nwt = math.ceil(cs / P)             # 98 weight row tiles
    scol = nwt * P                      # 12544 zero-padded width
    nchunk = math.ceil(scol / chunk)    # 7
    nbatch = math.ceil(nwt / wbatch)    # 7
    c_total = cs * num_cores
    ncw = max(nb, wbatch)

    kappa_m1 = float(np.exp(-S * MARG) - 1.0)
    const = float(S + (1.0 - EPS) * S * MARG + EPS * S * MARG / c_total)
    # DVE Schraudolph: y = raw*(K*S*invF) + (16256 - K*S - C_CORR)
    bimm = float(16256.0 - K_SCHR * S - C_CORR)

    nc = bacc.Bacc(
        "TRN2", target_bir_lowering=False, debug=False, num_devices=num_cores
    )

    f_ext = nc.dram_tensor("f", [b_sz, D], f32, kind="ExternalInput")
    w_ext = nc.dram_tensor("w", [cs, D], f32, kind="ExternalInput")
    lab_ext = nc.dram_tensor("lab", [P, nb], i32, kind="ExternalInput")
    coff_ext = nc.dram_tensor("coff", [P, 1], f32, kind="ExternalInput")
    id_ext = nc.dram_tensor("id32", [P, P], f32, kind="ExternalInput")
    out_ext = nc.dram_tensor("out", [1, 1], f32, kind="ExternalOutput")

    with tile.TileContext(nc) as tc:
        with (
            tc.tile_pool(name="consts", bufs=1) as consts,
            tc.tile_pool(name="persist", bufs=1) as persist,
            tc.tile_pool(name="wa", bufs=3) as wap,
            tc.tile_pool(name="wsc", bufs=3) as wscp,
            tc.tile_pool(name="wlab", bufs=max(nb, 2)) as wlabp,
            tc.tile_pool(name="small", bufs=3) as smallp,
            tc.tile_pool(name="i16p", bufs=3) as i16p,
            tc.tile_pool(name="psum_mm", bufs=2, space="PSUM") as psmm,
            tc.tile_pool(name="dram", bufs=1, space="DRAM") as dramp,
        ):
            # ---- constants ------------------------------------------------
            ident32 = consts.tile([P, P], f32, name="ident32")
            nc.scalar.dma_start(out=ident32[:], in_=id_ext[:, :])
            ident = consts.tile([P, P], bf16, name="ident")
            nc.vector.tensor_copy(out=ident[:], in_=ident32[:])
            bias_m30 = consts.tile([P, 1], f32, name="bias_m30")
            nc.vector.memset(bias_m30[:], -S)
            magic = consts.tile([P, ncw], i32, name="magic")
            nc.vector.memset(magic[:], MAGIC)
            onei = consts.tile([P, ncw], i32, name="onei")
            nc.vector.memset(onei[:], 1)

            def rsqrt(ssq_ap, inv_ap, n):
                """inv = 1/sqrt(ssq): quake seed + 2 Newton steps, DVE only."""
                half = smallp.tile([P, n], i32, tag="nrm_h", name="nrm_h")
                y0i = smallp.tile([P, n], i32, tag="nrm_yi", name="nrm_yi")
                t = smallp.tile([P, n], f32, tag="nrm_t", name="nrm_t")
                y1 = smallp.tile([P, n], f32, tag="nrm_y1", name="nrm_y1")
                nc.vector.tensor_tensor(
                    out=half[:], in0=ssq_ap.bitcast(i32), in1=onei[:, 0:n],
                    op=OP.arith_shift_right,
                )
                nc.vector.tensor_tensor(
                    out=y0i[:], in0=magic[:, 0:n], in1=half[:], op=OP.subtract
                )
                y0 = y0i[:].bitcast(f32)
                nc.vector.tensor_tensor(out=t[:], in0=y0, in1=y0, op=OP.mult)
                nc.vector.tensor_tensor(out=t[:], in0=t[:], in1=ssq_ap, op=OP.mult)
                nc.vector.tensor_scalar(
                    out=t[:], in0=t[:], scalar1=-0.5, scalar2=1.5,
                    op0=OP.mult, op1=OP.add,
                )
                nc.vector.tensor_tensor(out=y1[:], in0=y0, in1=t[:], op=OP.mult)
                nc.vector.tensor_tensor(out=t[:], in0=y1[:], in1=y1[:], op=OP.mult)
                nc.vector.tensor_tensor(out=t[:], in0=t[:], in1=ssq_ap, op=OP.mult)
                nc.vector.tensor_scalar(
                    out=t[:], in0=t[:], scalar1=-0.5, scalar2=1.5,
                    op0=OP.mult, op1=OP.add,
                )
                nc.vector.tensor_tensor(out=inv_ap, in0=y1[:], in1=t[:], op=OP.mult)

            # ---- persistent SBUF state ------------------------------------
            wT = persist.tile([P, scol], bf16, name="wT")
            fa = persist.tile([P, nb * P], f32, name="fa")
            fcast = persist.tile([P, nb * P], bf16, name="fcast")
            fT = persist.tile([P, nb * P], bf16, name="fT")
            ssqF = persist.tile([P, nb], f32, name="ssqF")
            invF = persist.tile([P, nb], f32, name="invF")
            SinvF = persist.tile([P, nb], f32, name="SinvF")
            KSinvF = persist.tile([P, nb], f32, name="KSinvF")
            ssqW = persist.tile([P, nwt], f32, name="ssqW")
            invW = persist.tile([P, nwt], f32, name="invW")
            ssqL = persist.tile([P, nb], f32, name="ssqL")
            invL = persist.tile([P, nb], f32, name="invL")
            ZP = persist.tile([P, nb * nchunk], f32, name="ZP")
            dots = persist.tile([P, nb], f32, name="dots")
            labi = persist.tile([P, nb], i32, name="labi")
            labf = persist.tile([P, nb], f32, name="labf")
            locc = persist.tile([P, nb], f32, name="locc")
            mask = persist.tile([P, nb], f32, name="mask")
            loci = persist.tile([P, nb], i32, name="loci")
            coff = persist.tile([P, 1], f32, name="coff")
            s32parts = persist.tile([P, nbatch], f32, name="s32parts")
            s32tot = persist.tile([P, 1], f32, name="s32tot")
            s32col = persist.tile([P, 1], bf16, name="s32col")
            ccZTL = persist.tile([P, 3 * nb], f32, name="ccZTL")
            RZTL = persist.tile([P, 3 * nb], f32, name="RZTL")
            wbf_d = dramp.tile([scol, D], bf16, name="wbf_d")
            if scol > cs:
                zpad = consts.tile([P, D], bf16, name="zpad")
                nc.vector.memset(zpad[:], 0.0)
                nc.gpsimd.dma_start(
                    out=wbf_d[cs:scol, :], in_=zpad[0 : scol - cs, :]
                )

            # ---- input DMAs ----------------------------------------------
            def issue_wbatch_dma(k):
                r0 = k * wbatch * P
                rows = min(wbatch * P, cs - r0)
                full_t = rows // P
                rem = rows - full_t * P
                nt = full_t + (1 if rem else 0)
                wa = wap.tile([P, wbatch * P], f32, tag="wa", name=f"wa{k}")
                if rem:
                    nc.vector.memset(wa[:, full_t * P : (full_t + 1) * P], 1.0)
                if full_t:
                    nc.sync.dma_start(
                        out=wa[:, 0 : full_t * P].rearrange("p (t d) -> p t d", d=D),
                        in_=w_ext[r0 : r0 + full_t * P, :].rearrange(
                            "(t p) d -> p t d", p=P
                        ),
                    )
                if rem:
                    nc.sync.dma_start(
                        out=wa[0:rem, full_t * P : full_t * P + D],
                        in_=w_ext[r0 + full_t * P : r0 + rows, :],
                    )
                return (wa, nt, r0, rows, full_t, rem)

            was = {}
            was[0] = issue_wbatch_dma(0)  # batch 0 first on the sync queue
            was[1] = issue_wbatch_dma(1)

            # f + small inputs on the scalar queue (parallel with w)
            nc.scalar.dma_start(
                out=fa[:].rearrange("p (b d) -> p b d", d=D),
                in_=f_ext[:, :].rearrange("(p b) d -> p b d", b=nb),
            )
            nc.scalar.dma_start(out=labi[:], in_=lab_ext[:, :])
            nc.scalar.dma_start(out=coff[:], in_=coff_ext[:, :])

            def prep_wbatch(k):
                """normalize+cast (Pool) -> transpose into wT; s32 partial."""
                wa, nt, r0, rows, full_t, rem = was[k]
                scrb = smallp.tile([P, wbatch * P], f32, tag="scrb", name="scrb")
                nc.gpsimd.tensor_tensor(
                    out=scrb[:, 0 : nt * P], in0=wa[:, 0 : nt * P],
                    in1=wa[:, 0 : nt * P], op=OP.mult,
                )
                nc.vector.tensor_reduce(
                    out=ssqW[:, k * wbatch : k * wbatch + nt],
                    in_=scrb[:, 0 : nt * P].rearrange("p (t d) -> p t d", d=P),
                    axis=mybir.AxisListType.X,
                    op=OP.add,
                )
                rsqrt(ssqW[:, k * wbatch : k * wbatch + nt],
                      invW[:, k * wbatch : k * wbatch + nt], nt)
                wscb = wscp.tile([P, wbatch * P], bf16, tag="wsc", name="wscb")
                for t in range(nt):
                    gi = k * wbatch + t
                    nc.gpsimd.tensor_scalar(
                        out=wscb[:, t * P : (t + 1) * P],
                        in0=wa[:, t * P : (t + 1) * P],
                        scalar1=invW[:, gi : gi + 1], scalar2=None, op0=OP.mult,
                    )
                if k < PE_TP:
                    for t in range(nt):
                        gi = k * wbatch + t
                        tpp = psmm.tile([P, P], bf16, tag="pm", name="tp_w")
                        nc.tensor.transpose(
                            out=tpp[:], in_=wscb[:, t * P : (t + 1) * P],
                            identity=ident[:],
                        )
                        nc.vector.tensor_copy(
                            out=wT[:, gi * P : (gi + 1) * P], in_=tpp[:]
                        )
                    if rem:
                        # zero pad columns (pad rows carried memset 1.0)
                        nc.vector.memset(
                            wT[:, r0 + rows : r0 + nt * P], 0.0
                        )
                else:
                    if full_t:
                        nc.gpsimd.dma_start(
                            out=wbf_d[r0 : r0 + full_t * P, :].rearrange(
                                "(t p) d -> p t d", p=P
                            ),
                            in_=wscb[:, 0 : full_t * P].rearrange(
                                "p (t d) -> p t d", d=D
                            ),
                        )
                    if rem:
                        nc.gpsimd.dma_start(
                            out=wbf_d[r0 + full_t * P : r0 + rows, :],
                            in_=wscb[0:rem, full_t * P : full_t * P + D],
                        )
                    bend = min(wbatch * P, scol - r0)
                    nc.sync.dma_start_transpose(
                        out=wT[:, r0 : r0 + bend], in_=wbf_d[r0 : r0 + bend, :]
                    )
                # s32 partial: sum_c w_hat_c over this batch's classes
                bend = min(wbatch * P, scol - r0)
                nc.vector.tensor_reduce(
                    out=s32parts[:, k : k + 1],
                    in_=wT[:, r0 : r0 + bend],
                    axis=mybir.AxisListType.X,
                    op=OP.add,
                )

            prep_wbatch(0)

            # ---- f prep: ssq only; normalization folded into exp scale ----
            scrF = smallp.tile([P, nb * P], f32, tag="scrb", name="scrF")
            nc.gpsimd.tensor_tensor(out=scrF[:], in0=fa[:], in1=fa[:], op=OP.mult)
            nc.vector.tensor_reduce(
                out=ssqF[:, 0:nb],
                in_=scrF[:].rearrange("p (b d) -> p b d", d=P),
                axis=mybir.AxisListType.X,
                op=OP.add,
            )
            rsqrt(ssqF[:, 0:nb], invF[:, 0:nb], nb)
            nc.vector.tensor_scalar(
                out=SinvF[:], in0=invF[:], scalar1=S, scalar2=None, op0=OP.mult
            )
            nc.vector.tensor_scalar(
                out=KSinvF[:], in0=invF[:], scalar1=float(K_SCHR * S),
                scalar2=None, op0=OP.mult,
            )
            nc.gpsimd.tensor_copy(out=fcast[:], in_=fa[:])
            for b in range(nb):
                sl = slice(b * P, (b + 1) * P)
                tp = psmm.tile([P, P], bf16, tag="pm", name="tp_f")
                nc.tensor.transpose(out=tp[:], in_=fcast[:, sl], identity=ident[:])
                nc.vector.tensor_copy(out=fT[:, sl], in_=tp[:])

            # ---- label localization ---------------------------------------
            nc.vector.tensor_copy(out=labf[:], in_=labi[:])
            nc.vector.tensor_scalar(
                out=locc[:], in0=labf[:], scalar1=coff[:, 0:1], scalar2=0.0,
                op0=OP.subtract, op1=OP.max,
            )
            nc.vector.tensor_scalar(
                out=locc[:], in0=locc[:], scalar1=float(cs - 1), scalar2=None,
                op0=OP.min,
            )
            scrm = smallp.tile([P, nb], f32, tag="scrm", name="scrm")
            nc.vector.tensor_scalar(
                out=scrm[:], in0=labf[:], scalar1=coff[:, 0:1], scalar2=None,
                op0=OP.subtract,
            )
            nc.vector.tensor_tensor(
                out=mask[:], in0=scrm[:], in1=locc[:], op=OP.is_equal
            )
            nc.vector.tensor_copy(out=loci[:], in_=locc[:])

            # ---- main loop ------------------------------------------------
            wlabs = []

            def issue_wlab(b):
                wlab = wlabp.tile([P, D], f32, tag="wlab", name=f"wlab{b}")
                wlabs.append(wlab)
                nc.gpsimd.indirect_dma_start(
                    out=wlab[:],
                    out_offset=None,
                    in_=w_ext[:, :],
                    in_offset=bass.IndirectOffsetOnAxis(ap=loci[:, b : b + 1], axis=0),
                )

            for cc in range(nchunk):
                if cc + 2 < nbatch:
                    was[cc + 2] = issue_wbatch_dma(cc + 2)
                if cc < 4:
                    for b in range(cc * 4, cc * 4 + 4):
                        issue_wlab(b)
                if cc + 1 < nbatch:
                    prep_wbatch(cc + 1)
                if cc == nchunk - 1:
                    # s32 complete after batch-6 prep: build bf16 column + T
                    nc.vector.tensor_reduce(
                        out=s32tot[:], in_=s32parts[:],
                        axis=mybir.AxisListType.X, op=OP.add,
                    )
                    nc.vector.tensor_copy(out=s32col[:], in_=s32tot[:])
                    for b in range(nb):
                        ts = psmm.tile([P, 16], f32, tag="pm", name="ts")
                        nc.tensor.matmul(
                            out=ts[:, 0:1],
                            lhsT=fT[:, b * P : (b + 1) * P],
                            rhs=s32col[:, 0:1],
                            start=True, stop=True,
                        )
                        nc.vector.tensor_scalar(
                            out=ccZTL[:, nb + b : nb + b + 1], in0=ts[:, 0:1],
                            scalar1=SinvF[:, b : b + 1], scalar2=None, op0=OP.mult,
                        )

                c0 = cc * chunk
                cw = min(chunk, scol - c0)
                dpos = dve_positions(ND_BY_CC[cc])
                for b in range(nb):
                    lhs = fT[:, b * P : (b + 1) * P]
                    pm = psmm.tile([P, chunk], f32, tag="pm", name="pm")
                    for sgi in range(math.ceil(cw / 512)):
                        n0 = sgi * 512
                        nn = min(512, cw - n0)
                        nc.tensor.matmul(
                            out=pm[:, n0 : n0 + nn],
                            lhsT=lhs,
                            rhs=wT[:, c0 + n0 : c0 + n0 + nn],
                            start=True, stop=True,
                        )
                    zcol = ZP[:, b * nchunk + cc : b * nchunk + cc + 1]
                    if b in dpos:
                        t16 = i16p.tile([P, chunk], i16, tag="t16", name="t16")
                        nc.vector.tensor_scalar(
                            out=t16[:, 0:cw], in0=pm[:, 0:cw],
                            scalar1=KSinvF[:, b : b + 1], scalar2=bimm,
                            op0=OP.mult, op1=OP.add,
                        )
                        nc.vector.tensor_reduce(
                            out=zcol, in_=t16[:, 0:cw].bitcast(bf16),
                            axis=mybir.AxisListType.X, op=OP.add,
                        )
                    else:
                        nc.scalar.activation(
                            out=pm[:, 0:cw], in_=pm[:, 0:cw], func=FT.Exp,
                            bias=bias_m30[:, 0:1], scale=SinvF[:, b : b + 1],
                            accum_out=zcol,
                        )

            # ---- label-dot columns ----------------------------------------
            for b in range(nb):
                scr = smallp.tile([P, P], f32, tag="ttr_scr", name="ttr_scr")
                nc.vector.scalar_tensor_tensor(
                    out=scr[:], in0=wlabs[b][:], scalar=1.0, in1=wlabs[b][:],
                    op0=OP.mult, op1=OP.mult, accum_out=ssqL[:, b : b + 1],
                )
            rsqrt(ssqL[:, 0:nb], invL[:, 0:nb], nb)
            for b in range(nb):
                scr = smallp.tile([P, P], f32, tag="ttr_scr", name="ttr_scr")
                nc.vector.scalar_tensor_tensor(
                    out=scr[:], in0=wlabs[b][:], scalar=invL[:, b : b + 1],
                    in1=fa[:, b * P : (b + 1) * P], op0=OP.mult, op1=OP.mult,
                    accum_out=dots[:, b : b + 1],
                )
            scrl = smallp.tile([P, nb], f32, tag="scrm", name="scrl")
            nc.vector.tensor_tensor(out=scrl[:], in0=dots[:], in1=mask[:], op=OP.mult)
            nc.vector.tensor_tensor(
                out=ccZTL[:, 2 * nb : 3 * nb], in0=scrl[:], in1=SinvF[:], op=OP.mult
            )

            # ---- Z columns ------------------------------------------------
            nc.vector.tensor_reduce(
                out=ccZTL[:, 0:nb],
                in_=ZP[:].rearrange("p (b c) -> p b c", c=nchunk),
                axis=mybir.AxisListType.X,
                op=OP.add,
            )

            # ---- AllReduce(add) of [Z|T|L] --------------------------------
            cz_in = dramp.tile([P, 3 * nb], f32, name="cz_in")
            cz_out = dramp.tile([P, 3 * nb], f32, name="cz_out")
            nc.sync.dma_start(out=cz_in[:], in_=ccZTL[:])
            nc.gpsimd.collective_compute(
                "AllReduce",
                OP.add,
                replica_groups=[list(range(num_cores))],
                ins=[cz_in.opt()],
                outs=[cz_out.opt()],
            )
            nc.sync.dma_start(out=RZTL[:], in_=cz_out[:])

            # ---- final loss -----------------------------------------------
            Zg = RZTL[:, 0:nb]
            Tg = RZTL[:, nb : 2 * nb]
            Lg = RZTL[:, 2 * nb : 3 * nb]
            expL = smallp.tile([P, nb], f32, tag="fin", name="expL")
            zadj = smallp.tile([P, nb], f32, tag="fin2", name="zadj")
            lnz = smallp.tile([P, nb], f32, tag="fin3", name="lnz")
            t1 = smallp.tile([P, nb], f32, tag="fin4", name="t1")
            losscol = smallp.tile([P, 1], f32, tag="fin6", name="losscol")
            outsb = smallp.tile([1, 1], f32, tag="fin8", name="outsb")

            nc.scalar.activation(
                out=expL[:], in_=Lg, func=FT.Exp, bias=bias_m30[:, 0:1], scale=1.0
            )
            nc.vector.scalar_tensor_tensor(
                out=zadj[:], in0=expL[:], scalar=kappa_m1, in1=Zg,
                op0=OP.mult, op1=OP.add,
            )
            # DVE bitcast-ln: ln(x) ~= (bits*2^-23 - 126.94269504)*ln2
            nc.vector.tensor_copy(out=lnz[:], in_=zadj[:].bitcast(i32))
            nc.vector.tensor_scalar(
                out=lnz[:], in0=lnz[:], scalar1=float(LN2 / (1 << 23)),
                scalar2=float(-126.94269504 * LN2), op0=OP.mult, op1=OP.add,
            )
            nc.vector.scalar_tensor_tensor(
                out=t1[:], in0=Lg, scalar=-(1.0 - EPS), in1=lnz[:],
                op0=OP.mult, op1=OP.add,
            )
            nc.vector.scalar_tensor_tensor(
                out=t1[:], in0=Tg, scalar=-EPS / c_total, in1=t1[:],
                op0=OP.mult, op1=OP.add,
            )
            nc.vector.tensor_reduce(
                out=losscol[:], in_=t1[:], axis=mybir.AxisListType.X, op=OP.add
            )
            ones_ap = nc.const_aps.aps[(f32, 1.0)]
            pfin = psmm.tile([P, 16], f32, tag="pm", name="pfin")
            nc.tensor.matmul(
                out=pfin[0:1, 0:1], lhsT=losscol[:, 0:1], rhs=ones_ap[:, 0:1],
                start=True, stop=True,
            )
            nc.vector.tensor_scalar(
                out=outsb[:], in0=pfin[0:1, 0:1], scalar1=1.0 / b_sz,
                scalar2=const, op0=OP.mult, op1=OP.add,
            )
            nc.sync.dma_start(out=out_ext[:, :], in_=outsb[:])

    nc.compile()
    return nc


def make_in_maps(features, labels, weight, num_cores=NCORES, b_sz=B, cs=CS):
    nb = b_sz // P
    f = np.ascontiguousarray(np.asarray(features, dtype=np.float32))
    lab = np.ascontiguousarray(np.asarray(labels).astype(np.int32).reshape(P, nb))
    w = np.asarray(weight, dtype=np.float32)
    in_maps = []
    for k in range(num_cores):
        in_maps.append(
            {
                "f": f,
                "w": np.ascontiguousarray(w[k * cs : (k + 1) * cs]),
                "lab": lab,
                "coff": np.full((P, 1), k * cs, dtype=np.float32),
                "id32": np.eye(P, dtype=np.float32),
            }
        )
    return in_maps


_NC_CACHE = {}


def kernel(features, labels, weight):
    from concourse.bass_utils import run_bass_kernel_spmd

    if "nc" not in _NC_CACHE:
        _NC_CACHE["nc"] = build_graph()
    nc = _NC_CACHE["nc"]
    in_maps = make_in_maps(features, labels, weight)
    res = run_bass_kernel_spmd(nc, in_maps, core_ids=list(range(NCORES)))
    return np.float32(res.results[0]["out"][0, 0])


# revision 21
# speedup vs baseline: 1.5426x; 1.5426x over previous
"""AM-Softmax loss (margin=0.3, scale=30, label smoothing 0.1) on 8 TRN2 cores.

Vocab-parallel: classifier weight (C,d) sharded along C across 8 cores.

Main restructure vs the 314us baseline: the 25.6M-element exp+sum per core
(the ACT-engine bottleneck at ~154 G elem/s) is split between ACT (native
Exp with free accumulator) and DVE (Schraudolph bit-trick exp: one
fused mul+add with f32->i16 convert, then a bf16-bitcast tensor_reduce at
the 16-bit 2x/4x DVE rate).  Feature normalization is folded into the
per-sample (per-partition) activation scale / convert scale, so f is never
normalized explicitly; weight normalization is folded into the f32->bf16
cast on the Pool engine (tensor_scalar by 1/|w| per class row).

  loss_i = lse_i - 0.9*(l_i - 9) - (0.1/C)*(T_i - 9)
  lse_i  = 30 + ln(Z_i + (e^-9 - 1) * e^(l_i - 30))
  Z_i    = sum_c exp(30*cos_ic - 30)     (ACT exact / DVE bit-trick chunks)
  T_i    = 30 * invF_i * (f_i . s),  s = sum_c w_hat_c  (per-batch DVE
           X-reduce of wT accumulated into one [P,1] column)
  l_i    = 30 * f_hat_i . w_hat_{y_i}    (indirect-DMA gather on owner)

The w pipeline is software-pipelined with the main loop (batch k+2 DMA /
batch k+1 normalize+transpose / chunk k matmul+exp).  One AllReduce(add)
on packed [Z|T|L] at the end replaces the baseline's two AllGathers plus
local rank-reduce; the final log is a DVE bitcast-ln (no ACT table load).
"""

import math

import numpy as np

import concourse.bass as bass
import concourse.bacc as bacc
import concourse.mybir as mybir
from concourse import tile

P = 128
B, D, C = 2048, 128, 100000
NCORES = 8
CS = C // NCORES
S, MARG, EPS = 30.0, 0.3, 0.1

f32 = mybir.dt.float32
bf16 = mybir.dt.bfloat16
i16 = mybir.dt.int16
i32 = mybir.dt.int32
FT = mybir.ActivationFunctionType
OP = mybir.AluOpType

MAGIC = 0x5F3759DF
K_SCHR = 128.0 / math.log(2.0)     # bf16 exponent scale
C_CORR = 7.5                       # Schraudolph bias calibration
LN2 = math.log(2.0)

# ---- tunables --------------------------------------------------------------
CHUNK = 2048          # psum chunk columns (4 banks; x2 bufs = full PSUM)
PE_TP = 2             # weight batches transposed on PE (rest via DMA xbar)
ND_BY_CC = [4, 4, 4, 5, 5, 5, 5]   # DVE-exp chunks per 16-chunk group
NP_BY_CC = [0, 0, 0, 0, 0, 0, 0]   # Pool-exp chunks (gpsimd can't read PSUM)
POOL_FOLD = True  # Pool does the bf16 halves-fold for DVE chunks


def spread_positions(n, avoid=(), width=16):
    """Spread n positions evenly over width b-slots, avoiding taken slots."""
    picks = []
    if n:
        cand = [int(round(i * width / n)) % width for i in range(n)]
        taken = set(avoid)
        for c in cand:
            while c in taken:
                c = (c + 1) % width
            picks.append(c)
            taken.add(c)
    return set(picks)


def build_graph(num_cores=NCORES, b_sz=B, cs=CS, chunk=CHUNK, wbatch=16):
    nb = b_sz // P                      # 16 B tiles
    nwt = math.ceil(cs / P)             # 98 weight row tiles
    scol = nwt * P                      # 12544 zero-padded width
    nchunk = math.ceil(scol / chunk)    # 7
    nbatch = math.ceil(nwt / wbatch)    # 7
    c_total = cs * num_cores
    ncw = max(nb, wbatch)

    kappa_m1 = float(np.exp(-S * MARG) - 1.0)
    const = float(S + (1.0 - EPS) * S * MARG + EPS * S * MARG / c_total)
    # DVE Schraudolph: y = raw*(K*S*invF) + (16256 - K*S - C_CORR)
    bimm = float(16256.0 - K_SCHR * S - C_CORR)

    nc = bacc.Bacc(
        "TRN2", target_bir_lowering=False, debug=False, num_devices=num_cores
    )

    f_ext = nc.dram_tensor("f", [b_sz, D], f32, kind="ExternalInput")
    w_ext = nc.dram_tensor("w", [cs, D], f32, kind="ExternalInput")
    lab_ext = nc.dram_tensor("lab", [P, nb], i32, kind="ExternalInput")
    coff_ext = nc.dram_tensor("coff", [P, 1], f32, kind="ExternalInput")
    id_ext = nc.dram_tensor("id32", [P, P], f32, kind="ExternalInput")
    out_ext = nc.dram_tensor("out", [1, 1], f32, kind="ExternalOutput")

    with tile.TileContext(nc) as tc:
        with (
            tc.tile_pool(name="consts", bufs=1) as consts,
            tc.tile_pool(name="persist", bufs=1) as persist,
            tc.tile_pool(name="wa", bufs=3) as wap,
            tc.tile_pool(name="wsc", bufs=3) as wscp,
            tc.tile_pool(name="wlab", bufs=max(nb, 2)) as wlabp,
            tc.tile_pool(name="small", bufs=3) as smallp,
            tc.tile_pool(name="i16p", bufs=3) as i16p,
            tc.tile_pool(name="psum_mm", bufs=2, space="PSUM") as psmm,
            tc.tile_pool(name="dram", bufs=1, space="DRAM") as dramp,
        ):
            # ---- constants ------------------------------------------------
            ident32 = consts.tile([P, P], f32, name="ident32")
            nc.scalar.dma_start(out=ident32[:], in_=id_ext[:, :])
            ident = consts.tile([P, P], bf16, name="ident")
            nc.vector.tensor_copy(out=ident[:], in_=ident32[:])
            bias_m30 = consts.tile([P, 1], f32, name="bias_m30")
            nc.vector.memset(bias_m30[:], -S)
            magic = consts.tile([P, ncw], i32, name="magic")
            nc.vector.memset(magic[:], MAGIC)
            onei = consts.tile([P, ncw], i32, name="onei")
            nc.vector.memset(onei[:], 1)
            onescol = consts.tile([P, 1], bf16, name="onescol")
            nc.vector.memset(onescol[:], 1.0)

            def rsqrt(ssq_ap, inv_ap, n):
                """inv = 1/sqrt(ssq): quake seed + 2 Newton steps, DVE only."""
                half = smallp.tile([P, n], i32, tag="nrm_h", name="nrm_h")
                y0i = smallp.tile([P, n], i32, tag="nrm_yi", name="nrm_yi")
                t = smallp.tile([P, n], f32, tag="nrm_t", name="nrm_t")
                y1 = smallp.tile([P, n], f32, tag="nrm_y1", name="nrm_y1")
                nc.vector.tensor_tensor(
                    out=half[:], in0=ssq_ap.bitcast(i32), in1=onei[:, 0:n],
                    op=OP.arith_shift_right,
                )
                nc.vector.tensor_tensor(
                    out=y0i[:], in0=magic[:, 0:n], in1=half[:], op=OP.subtract
                )
                y0 = y0i[:].bitcast(f32)
                nc.vector.tensor_tensor(out=t[:], in0=y0, in1=y0, op=OP.mult)
                nc.vector.tensor_tensor(out=t[:], in0=t[:], in1=ssq_ap, op=OP.mult)
                nc.vector.tensor_scalar(
                    out=t[:], in0=t[:], scalar1=-0.5, scalar2=1.5,
                    op0=OP.mult, op1=OP.add,
                )
                nc.vector.tensor_tensor(out=y1[:], in0=y0, in1=t[:], op=OP.mult)
                nc.vector.tensor_tensor(out=t[:], in0=y1[:], in1=y1[:], op=OP.mult)
                nc.vector.tensor_tensor(out=t[:], in0=t[:], in1=ssq_ap, op=OP.mult)
                nc.vector.tensor_scalar(
                    out=t[:], in0=t[:], scalar1=-0.5, scalar2=1.5,
                    op0=OP.mult, op1=OP.add,
                )
                nc.vector.tensor_tensor(out=inv_ap, in0=y1[:], in1=t[:], op=OP.mult)

            # ---- persistent SBUF state ------------------------------------
            wT = persist.tile([P, scol], bf16, name="wT")
            fa = persist.tile([P, nb * P], f32, name="fa")
            fcast = persist.tile([P, nb * P], bf16, name="fcast")
            fT = persist.tile([P, nb * P], bf16, name="fT")
            ssqF = persist.tile([P, nb], f32, name="ssqF")
            invF = persist.tile([P, nb], f32, name="invF")
            SinvF = persist.tile([P, nb], f32, name="SinvF")
            KSinvF = persist.tile([P, nb], f32, name="KSinvF")
            ssqW = persist.tile([P, nwt], f32, name="ssqW")
            invW = persist.tile([P, nwt], f32, name="invW")
            ssqL = persist.tile([P, nb], f32, name="ssqL")
            invL = persist.tile([P, nb], f32, name="invL")
            ZP = persist.tile([P, nb * nchunk], f32, name="ZP")
            dots = persist.tile([P, nb], f32, name="dots")
            labi = persist.tile([P, nb], i32, name="labi")
            labf = persist.tile([P, nb], f32, name="labf")
            locc = persist.tile([P, nb], f32, name="locc")
            mask = persist.tile([P, nb], f32, name="mask")
            loci = persist.tile([P, nb], i32, name="loci")
            coff = persist.tile([P, 1], f32, name="coff")
            s32row = persist.tile([1, P], f32, name="s32row")
            s32rb = persist.tile([1, P], bf16, name="s32rb")
            s32col = persist.tile([P, 1], bf16, name="s32col")
            wlab = persist.tile([P, nb * D], f32, name="wlab")
            dotsr = persist.tile([P, nb], f32, name="dotsr")
            ccZTL = persist.tile([P, 3 * nb], f32, name="ccZTL")
            RZTL = persist.tile([P, 3 * nb], f32, name="RZTL")
            wbf_d = dramp.tile([scol, D], bf16, name="wbf_d")
            if scol > cs:
                zpad = consts.tile([P, D], bf16, name="zpad")
                nc.vector.memset(zpad[:], 0.0)
                nc.gpsimd.dma_start(
                    out=wbf_d[cs:scol, :], in_=zpad[0 : scol - cs, :]
                )

            # ---- input DMAs ----------------------------------------------
            def issue_wbatch_dma(k):
                r0 = k * wbatch * P
                rows = min(wbatch * P, cs - r0)
                full_t = rows // P
                rem = rows - full_t * P
                nt = full_t + (1 if rem else 0)
                wa = wap.tile([P, wbatch * P], f32, tag="wa", name=f"wa{k}")
                if rem:
                    nc.vector.memset(wa[:, full_t * P : (full_t + 1) * P], 1.0)
                if full_t:
                    nc.sync.dma_start(
                        out=wa[:, 0 : full_t * P].rearrange("p (t d) -> p t d", d=D),
                        in_=w_ext[r0 : r0 + full_t * P, :].rearrange(
                            "(t p) d -> p t d", p=P
                        ),
                    )
                if rem:
                    nc.sync.dma_start(
                        out=wa[0:rem, full_t * P : full_t * P + D],
                        in_=w_ext[r0 + full_t * P : r0 + rows, :],
                    )
                return (wa, nt, r0, rows, full_t, rem)

            was = {}
            was[0] = issue_wbatch_dma(0)  # batch 0 first on the sync queue
            was[1] = issue_wbatch_dma(1)
            nc.vector.memset(s32row[:], 0.0)

            # f + small inputs on the scalar queue (parallel with w)
            nc.scalar.dma_start(
                out=fa[:].rearrange("p (b d) -> p b d", d=D),
                in_=f_ext[:, :].rearrange("(p b) d -> p b d", b=nb),
            )
            nc.scalar.dma_start(out=labi[:], in_=lab_ext[:, :])
            nc.scalar.dma_start(out=coff[:], in_=coff_ext[:, :])

            def prep_wbatch(k):
                """normalize+cast -> transpose into wT; s32 partial."""
                wa, nt, r0, rows, full_t, rem = was[k]
                scrb = smallp.tile([P, wbatch * P], f32, tag="scrb", name="scrb")
                nc.gpsimd.tensor_tensor(
                    out=scrb[:, 0 : nt * P], in0=wa[:, 0 : nt * P],
                    in1=wa[:, 0 : nt * P], op=OP.mult,
                )
                nc.vector.tensor_reduce(
                    out=ssqW[:, k * wbatch : k * wbatch + nt],
                    in_=scrb[:, 0 : nt * P].rearrange("p (t d) -> p t d", d=P),
                    axis=mybir.AxisListType.X,
                    op=OP.add,
                )
                rsqrt(ssqW[:, k * wbatch : k * wbatch + nt],
                      invW[:, k * wbatch : k * wbatch + nt], nt)
                # scale+cast in ONE DVE op: broadcast invW per tile block
                wscb = wscp.tile([P, wbatch * P], bf16, tag="wsc", name="wscb")
                nc.vector.scalar_tensor_tensor(
                    out=wscb[:, 0 : nt * P].rearrange("p (t d) -> p t d", d=P),
                    in0=wa[:, 0 : nt * P].rearrange("p (t d) -> p t d", d=P),
                    scalar=1.0,
                    in1=invW[:, k * wbatch : k * wbatch + nt].broadcast_to(
                        (P, nt, P)
                    ),
                    op0=OP.mult, op1=OP.mult,
                )
                if k < PE_TP:
                    for t in range(nt):
                        gi = k * wbatch + t
                        tpp = psmm.tile([P, P], bf16, tag="pm", name="tp_w")
                        nc.tensor.transpose(
                            out=tpp[:], in_=wscb[:, t * P : (t + 1) * P],
                            identity=ident[:],
                        )
                        nc.vector.tensor_copy(
                            out=wT[:, gi * P : (gi + 1) * P], in_=tpp[:]
                        )
                    if rem:
                        # zero pad columns (pad rows carried memset 1.0)
                        nc.vector.memset(
                            wT[:, r0 + rows : r0 + nt * P], 0.0
                        )
                else:
                    if full_t:
                        nc.gpsimd.dma_start(
                            out=wbf_d[r0 : r0 + full_t * P, :].rearrange(
                                "(t p) d -> p t d", p=P
                            ),
                            in_=wscb[:, 0 : full_t * P].rearrange(
                                "p (t d) -> p t d", d=D
                            ),
                        )
                    if rem:
                        nc.gpsimd.dma_start(
                            out=wbf_d[r0 + full_t * P : r0 + rows, :],
                            in_=wscb[0:rem, full_t * P : full_t * P + D],
                        )
                    bend = min(wbatch * P, scol - r0)
                    nc.sync.dma_start_transpose(
                        out=wT[:, r0 : r0 + bend], in_=wbf_d[r0 : r0 + bend, :]
                    )
                # s32 partial via PE: ones^T @ wscb tiles -> [1, D] row,
                # accumulated in psum across tiles, DVE-added into s32row
                s32p = psmm.tile([P, chunk], f32, tag="pm", name="s32p")
                for t in range(nt):
                    kp = P if (t < full_t or not rem) else rem
                    nc.tensor.matmul(
                        out=s32p[0:1, 0:D],
                        lhsT=onescol[0:kp, 0:1],
                        rhs=wscb[0:kp, t * P : t * P + D],
                        start=(t == 0), stop=(t == nt - 1),
                    )
                nc.vector.tensor_tensor(
                    out=s32row[:], in0=s32row[:], in1=s32p[0:1, 0:D], op=OP.add
                )

            prep_wbatch(0)

            # ---- f prep: ssq only; normalization folded into exp scale ----
            scrF = smallp.tile([P, nb * P], f32, tag="scrb", name="scrF")
            nc.gpsimd.tensor_tensor(out=scrF[:], in0=fa[:], in1=fa[:], op=OP.mult)
            nc.vector.tensor_reduce(
                out=ssqF[:, 0:nb],
                in_=scrF[:].rearrange("p (b d) -> p b d", d=P),
                axis=mybir.AxisListType.X,
                op=OP.add,
            )
            rsqrt(ssqF[:, 0:nb], invF[:, 0:nb], nb)
            nc.vector.tensor_scalar(
                out=SinvF[:], in0=invF[:], scalar1=S, scalar2=None, op0=OP.mult
            )
            nc.vector.tensor_scalar(
                out=KSinvF[:], in0=invF[:], scalar1=float(K_SCHR * S),
                scalar2=None, op0=OP.mult,
            )
            nc.vector.tensor_copy(out=fcast[:], in_=fa[:])
            for b in range(nb):
                sl = slice(b * P, (b + 1) * P)
                tp = psmm.tile([P, P], bf16, tag="pm", name="tp_f")
                nc.tensor.transpose(out=tp[:], in_=fcast[:, sl], identity=ident[:])
                nc.vector.tensor_copy(out=fT[:, sl], in_=tp[:])

            # ---- label localization ---------------------------------------
            nc.vector.tensor_copy(out=labf[:], in_=labi[:])
            nc.vector.tensor_scalar(
                out=locc[:], in0=labf[:], scalar1=coff[:, 0:1], scalar2=0.0,
                op0=OP.subtract, op1=OP.max,
            )
            nc.vector.tensor_scalar(
                out=locc[:], in0=locc[:], scalar1=float(cs - 1), scalar2=None,
                op0=OP.min,
            )
            scrm = smallp.tile([P, nb], f32, tag="scrm", name="scrm")
            nc.vector.tensor_scalar(
                out=scrm[:], in0=labf[:], scalar1=coff[:, 0:1], scalar2=None,
                op0=OP.subtract,
            )
            nc.vector.tensor_tensor(
                out=mask[:], in0=scrm[:], in1=locc[:], op=OP.is_equal
            )
            nc.vector.tensor_copy(out=loci[:], in_=locc[:])

            # ---- main loop ------------------------------------------------
            def issue_wlab(b):
                nc.gpsimd.indirect_dma_start(
                    out=wlab[:, b * D : (b + 1) * D],
                    out_offset=None,
                    in_=w_ext[:, :],
                    in_offset=bass.IndirectOffsetOnAxis(ap=loci[:, b : b + 1], axis=0),
                )

            for cc in range(nchunk):
                if cc + 2 < nbatch:
                    was[cc + 2] = issue_wbatch_dma(cc + 2)
                if cc < 4:
                    for b in range(cc * 4, cc * 4 + 4):
                        issue_wlab(b)
                if cc + 1 < nbatch:
                    prep_wbatch(cc + 1)
                if cc == nchunk - 1:
                    # s32 row complete after batch-6 prep: transpose to a
                    # [P,1] bf16 column via a K=1 matmul, then batched T
                    nc.vector.tensor_copy(out=s32rb[:], in_=s32row[:])
                    tsc = psmm.tile([P, chunk], f32, tag="pm", name="tsc")
                    nc.tensor.matmul(
                        out=tsc[0:P, 0:1], lhsT=s32rb[0:1, 0:P],
                        rhs=onescol[0:1, 0:1], start=True, stop=True,
                    )
                    nc.vector.tensor_copy(out=s32col[:], in_=tsc[0:P, 0:1])
                    tsall = psmm.tile([P, chunk], f32, tag="pm", name="tsall")
                    for b in range(nb):
                        nc.tensor.matmul(
                            out=tsall[:, b : b + 1],
                            lhsT=fT[:, b * P : (b + 1) * P],
                            rhs=s32col[:, 0:1],
                            start=True, stop=True,
                        )
                    nc.vector.tensor_tensor(
                        out=ccZTL[:, nb : 2 * nb], in0=tsall[:, 0:nb],
                        in1=SinvF[:], op=OP.mult,
                    )

                c0 = cc * chunk
                cw = min(chunk, scol - c0)
                dpos = spread_positions(ND_BY_CC[cc])
                ppos = spread_positions(NP_BY_CC[cc], avoid=dpos)
                for b in range(nb):
                    lhs = fT[:, b * P : (b + 1) * P]
                    pm = psmm.tile([P, chunk], f32, tag="pm", name="pm")
                    for sgi in range(math.ceil(cw / 512)):
                        n0 = sgi * 512
                        nn = min(512, cw - n0)
                        nc.tensor.matmul(
                            out=pm[:, n0 : n0 + nn],
                            lhsT=lhs,
                            rhs=wT[:, c0 + n0 : c0 + n0 + nn],
                            start=True, stop=True,
                        )
                    zcol = ZP[:, b * nchunk + cc : b * nchunk + cc + 1]
                    if b in dpos:
                        # DVE Schraudolph convert; Pool folds bf16 halves
                        # (SBUF->SBUF); DVE reduces the half-width result
                        t16 = i16p.tile([P, chunk], i16, tag="t16", name="t16")
                        nc.vector.tensor_scalar(
                            out=t16[:, 0:cw], in0=pm[:, 0:cw],
                            scalar1=KSinvF[:, b : b + 1], scalar2=bimm,
                            op0=OP.mult, op1=OP.add,
                        )
                        if POOL_FOLD and cw == chunk:
                            h = cw // 2
                            fold = i16p.tile([P, chunk // 2], bf16, tag="fold",
                                             name="fold")
                            nc.gpsimd.tensor_tensor(
                                out=fold[:, 0:h],
                                in0=t16[:, 0:h].bitcast(bf16),
                                in1=t16[:, h : 2 * h].bitcast(bf16),
                                op=OP.add,
                            )
                            nc.vector.tensor_reduce(
                                out=zcol, in_=fold[:, 0:h],
                                axis=mybir.AxisListType.X, op=OP.add,
                            )
                        else:
                            nc.vector.tensor_reduce(
                                out=zcol, in_=t16[:, 0:cw].bitcast(bf16),
                                axis=mybir.AxisListType.X, op=OP.add,
                            )
                    else:
                        nc.scalar.activation(
                            out=pm[:, 0:cw], in_=pm[:, 0:cw], func=FT.Exp,
                            bias=bias_m30[:, 0:1], scale=SinvF[:, b : b + 1],
                            accum_out=zcol,
                        )

            # ---- label-dot columns (batched [P, nb*D] ops) ----------------
            scrw = smallp.tile([P, nb * D], f32, tag="scrb", name="scrw")
            nc.gpsimd.tensor_tensor(
                out=scrw[:], in0=wlab[:], in1=wlab[:], op=OP.mult
            )
            nc.vector.tensor_reduce(
                out=ssqL[:, 0:nb],
                in_=scrw[:].rearrange("p (b d) -> p b d", d=D),
                axis=mybir.AxisListType.X, op=OP.add,
            )
            rsqrt(ssqL[:, 0:nb], invL[:, 0:nb], nb)
            scrw2 = smallp.tile([P, nb * D], f32, tag="scrb", name="scrw2")
            nc.gpsimd.tensor_tensor(
                out=scrw2[:], in0=wlab[:], in1=fa[:], op=OP.mult
            )
            nc.vector.tensor_reduce(
                out=dotsr[:],
                in_=scrw2[:].rearrange("p (b d) -> p b d", d=D),
                axis=mybir.AxisListType.X, op=OP.add,
            )
            nc.vector.tensor_tensor(out=dots[:], in0=dotsr[:], in1=invL[:], op=OP.mult)
            scrl = smallp.tile([P, nb], f32, tag="scrm", name="scrl")
            nc.vector.tensor_tensor(out=scrl[:], in0=dots[:], in1=mask[:], op=OP.mult)
            nc.vector.tensor_tensor(
                out=ccZTL[:, 2 * nb : 3 * nb], in0=scrl[:], in1=SinvF[:], op=OP.mult
            )

            # ---- Z columns ------------------------------------------------
            nc.vector.tensor_reduce(
                out=ccZTL[:, 0:nb],
                in_=ZP[:].rearrange("p (b c) -> p b c", c=nchunk),
                axis=mybir.AxisListType.X,
                op=OP.add,
            )

            # ---- AllReduce(add) of [Z|T|L] --------------------------------
            cz_in = dramp.tile([P, 3 * nb], f32, name="cz_in")
            cz_out = dramp.tile([P, 3 * nb], f32, name="cz_out")
            nc.sync.dma_start(out=cz_in[:], in_=ccZTL[:])
            nc.gpsimd.collective_compute(
                "AllReduce",
                OP.add,
                replica_groups=[list(range(num_cores))],
                ins=[cz_in.opt()],
                outs=[cz_out.opt()],
            )
            nc.sync.dma_start(out=RZTL[:], in_=cz_out[:])

            # ---- final loss -----------------------------------------------
            Zg = RZTL[:, 0:nb]
            Tg = RZTL[:, nb : 2 * nb]
            Lg = RZTL[:, 2 * nb : 3 * nb]
            expL = smallp.tile([P, nb], f32, tag="fin", name="expL")
            zadj = smallp.tile([P, nb], f32, tag="fin2", name="zadj")
            lnz = smallp.tile([P, nb], f32, tag="fin3", name="lnz")
            t1 = smallp.tile([P, nb], f32, tag="fin4", name="t1")
            losscol = smallp.tile([P, 1], f32, tag="fin6", name="losscol")
            outsb = smallp.tile([1, 1], f32, tag="fin8", name="outsb")

            nc.scalar.activation(
                out=expL[:], in_=Lg, func=FT.Exp, bias=bias_m30[:, 0:1], scale=1.0
            )
            nc.vector.scalar_tensor_tensor(
                out=zadj[:], in0=expL[:], scalar=kappa_m1, in1=Zg,
                op0=OP.mult, op1=OP.add,
            )
            # DVE bitcast-ln: ln(x) ~= (bits*2^-23 - 126.94269504)*ln2
            nc.vector.tensor_copy(out=lnz[:], in_=zadj[:].bitcast(i32))
            nc.vector.tensor_scalar(
                out=lnz[:], in0=lnz[:], scalar1=float(LN2 / (1 << 23)),
                scalar2=float(-126.94269504 * LN2), op0=OP.mult, op1=OP.add,
            )
            nc.vector.scalar_tensor_tensor(
                out=t1[:], in0=Lg, scalar=-(1.0 - EPS), in1=lnz[:],
                op0=OP.mult, op1=OP.add,
            )
            nc.vector.scalar_tensor_tensor(
                out=t1[:], in0=Tg, scalar=-EPS / c_total, in1=t1[:],
                op0=OP.mult, op1=OP.add,
            )
            nc.vector.tensor_reduce(
                out=losscol[:], in_=t1[:], axis=mybir.AxisListType.X, op=OP.add
            )
            ones_ap = nc.const_aps.aps[(f32, 1.0)]
            pfin = psmm.tile([P, 16], f32, tag="pm", name="pfin")
            nc.tensor.matmul(
                out=pfin[0:1, 0:1], lhsT=losscol[:, 0:1], rhs=ones_ap[:, 0:1],
                start=True, stop=True,
            )
            nc.vector.tensor_scalar(
                out=outsb[:], in0=pfin[0:1, 0:1], scalar1=1.0 / b_sz,
                scalar2=const, op0=OP.mult, op1=OP.add,
            )
            nc.sync.dma_start(out=out_ext[:, :], in_=outsb[:])

    nc.compile()
    return nc


def make_in_maps(features, labels, weight, num_cores=NCORES, b_sz=B, cs=CS):
    nb = b_sz // P
    f = np.ascontiguousarray(np.asarray(features, dtype=np.float32))
    lab = np.ascontiguousarray(np.asarray(labels).astype(np.int32).reshape(P, nb))
    w = np.asarray(weight, dtype=np.float32)
    in_maps = []
    for k in range(num_cores):
        in_maps.append(
            {
                "f": f,
                "w": np.ascontiguousarray(w[k * cs : (k + 1) * cs]),
                "lab": lab,
                "coff": np.full((P, 1), k * cs, dtype=np.float32),
                "id32": np.eye(P, dtype=np.float32),
            }
        )
    return in_maps


_NC_CACHE = {}


def kernel(features, labels, weight):
    from concourse.bass_utils import run_bass_kernel_spmd

    if "nc" not in _NC_CACHE:
        _NC_CACHE["nc"] = build_graph()
    nc = _NC_CACHE["nc"]
    in_maps = make_in_maps(features, labels, weight)
    res = run_bass_kernel_spmd(nc, in_maps, core_ids=list(range(NCORES)))
    return np.float32(res.results[0]["out"][0, 0])


# revision 26
# speedup vs baseline: 1.6960x; 1.0994x over previous
"""AM-Softmax loss (margin=0.3, scale=30, label smoothing 0.1) on 8 TRN2 cores.

Vocab-parallel: classifier weight (C,d) sharded along C across 8 cores.

Main restructure vs the 314us baseline: the 25.6M-element exp+sum per core
(the ACT-engine bottleneck at ~154 G elem/s) is split between ACT (native
Exp with free accumulator) and DVE (Schraudolph bit-trick exp: one
fused mul+add with f32->i16 convert, then a bf16-bitcast tensor_reduce at
the 16-bit 2x/4x DVE rate).  Feature normalization is folded into the
per-sample (per-partition) activation scale / convert scale, so f is never
normalized explicitly; weight normalization is folded into the f32->bf16
cast on the Pool engine (tensor_scalar by 1/|w| per class row).

  loss_i = lse_i - 0.9*(l_i - 9) - (0.1/C)*(T_i - 9)
  lse_i  = 30 + ln(Z_i + (e^-9 - 1) * e^(l_i - 30))
  Z_i    = sum_c exp(30*cos_ic - 30)     (ACT exact / DVE bit-trick chunks)
  T_i    = 30 * invF_i * (f_i . s),  s = sum_c w_hat_c  (per-batch DVE
           X-reduce of wT accumulated into one [P,1] column)
  l_i    = 30 * f_hat_i . w_hat_{y_i}    (indirect-DMA gather on owner)

The w pipeline is software-pipelined with the main loop (batch k+2 DMA /
batch k+1 normalize+transpose / chunk k matmul+exp).  One AllReduce(add)
on packed [Z|T|L] at the end replaces the baseline's two AllGathers plus
local rank-reduce; the final log is a DVE bitcast-ln (no ACT table load).
"""

import math

import numpy as np

import concourse.bass as bass
import concourse.bacc as bacc
import concourse.mybir as mybir
from concourse import tile

P = 128
B, D, C = 2048, 128, 100000
NCORES = 8
CS = C // NCORES
S, MARG, EPS = 30.0, 0.3, 0.1

f32 = mybir.dt.float32
bf16 = mybir.dt.bfloat16
i16 = mybir.dt.int16
i32 = mybir.dt.int32
FT = mybir.ActivationFunctionType
OP = mybir.AluOpType

MAGIC = 0x5F3759DF
K_SCHR = 128.0 / math.log(2.0)     # bf16 exponent scale
C_CORR = 7.5                       # Schraudolph bias calibration
LN2 = math.log(2.0)

# ---- tunables --------------------------------------------------------------
CHUNK = 1024          # psum chunk columns (2 banks; x4 bufs = full PSUM)
PE_TP = 2             # weight batches transposed on PE (rest via DMA xbar)
PSUM_BUFS = 4
ND_BY_CC = [4, 5, 5, 5, 5, 5, 5]   # DVE-exp chunks per 16-chunk half-group
DVE_FOLD = True       # DVE folds bf16 halves before the reduce


def spread_positions(n, avoid=(), width=16):
    """Spread n positions evenly over width b-slots, avoiding taken slots."""
    picks = []
    if n:
        cand = [int(round(i * width / n)) % width for i in range(n)]
        taken = set(avoid)
        for c in cand:
            while c in taken:
                c = (c + 1) % width
            picks.append(c)
            taken.add(c)
    return set(picks)


def build_graph(num_cores=NCORES, b_sz=B, cs=CS, chunk=CHUNK, wbatch=16):
    nb = b_sz // P                      # 16 B tiles
    nwt = math.ceil(cs / P)             # 98 weight row tiles
    scol = nwt * P                      # 12544 zero-padded width
    gcol = wbatch * P                   # 2048 columns per batch group
    nbatch = math.ceil(nwt / wbatch)    # 7
    nhalf = gcol // chunk               # chunks per batch group per b
    nchunk = math.ceil(scol / chunk)    # 13 ZP slots per b
    c_total = cs * num_cores
    ncw = max(nb, wbatch)

    kappa_m1 = float(np.exp(-S * MARG) - 1.0)
    const = float(S + (1.0 - EPS) * S * MARG + EPS * S * MARG / c_total)
    # DVE Schraudolph: y = raw*(K*S*invF) + (16256 - K*S - C_CORR)
    bimm = float(16256.0 - K_SCHR * S - C_CORR)

    nc = bacc.Bacc(
        "TRN2", target_bir_lowering=False, debug=False, num_devices=num_cores
    )

    f_ext = nc.dram_tensor("f", [b_sz, D], f32, kind="ExternalInput")
    w_ext = nc.dram_tensor("w", [cs, D], f32, kind="ExternalInput")
    lab_ext = nc.dram_tensor("lab", [P, nb], i32, kind="ExternalInput")
    coff_ext = nc.dram_tensor("coff", [P, 1], f32, kind="ExternalInput")
    id_ext = nc.dram_tensor("id32", [P, P], f32, kind="ExternalInput")
    out_ext = nc.dram_tensor("out", [1, 1], f32, kind="ExternalOutput")

    with tile.TileContext(nc) as tc:
        with (
            tc.tile_pool(name="consts", bufs=1) as consts,
            tc.tile_pool(name="persist", bufs=1) as persist,
            tc.tile_pool(name="wa", bufs=3) as wap,
            tc.tile_pool(name="wsc", bufs=3) as wscp,
            tc.tile_pool(name="wlab", bufs=max(nb, 2)) as wlabp,
            tc.tile_pool(name="small", bufs=3) as smallp,
            tc.tile_pool(name="i16p", bufs=3) as i16p,
            tc.tile_pool(name="psum_mm", bufs=PSUM_BUFS, space="PSUM") as psmm,
            tc.tile_pool(name="dram", bufs=1, space="DRAM") as dramp,
        ):
            # ---- constants ------------------------------------------------
            ident32 = consts.tile([P, P], f32, name="ident32")
            nc.scalar.dma_start(out=ident32[:], in_=id_ext[:, :])
            ident = consts.tile([P, P], bf16, name="ident")
            nc.vector.tensor_copy(out=ident[:], in_=ident32[:])
            bias_m30 = consts.tile([P, 1], f32, name="bias_m30")
            nc.vector.memset(bias_m30[:], -S)
            magic = consts.tile([P, ncw], i32, name="magic")
            nc.vector.memset(magic[:], MAGIC)
            onei = consts.tile([P, ncw], i32, name="onei")
            nc.vector.memset(onei[:], 1)
            onescol = consts.tile([P, 1], bf16, name="onescol")
            nc.vector.memset(onescol[:], 1.0)

            def rsqrt(ssq_ap, inv_ap, n):
                """inv = 1/sqrt(ssq): quake seed + 2 Newton steps, DVE only."""
                half = smallp.tile([P, n], i32, tag="nrm_h", name="nrm_h")
                y0i = smallp.tile([P, n], i32, tag="nrm_yi", name="nrm_yi")
                t = smallp.tile([P, n], f32, tag="nrm_t", name="nrm_t")
                y1 = smallp.tile([P, n], f32, tag="nrm_y1", name="nrm_y1")
                nc.vector.tensor_tensor(
                    out=half[:], in0=ssq_ap.bitcast(i32), in1=onei[:, 0:n],
                    op=OP.arith_shift_right,
                )
                nc.vector.tensor_tensor(
                    out=y0i[:], in0=magic[:, 0:n], in1=half[:], op=OP.subtract
                )
                y0 = y0i[:].bitcast(f32)
                nc.vector.tensor_tensor(out=t[:], in0=y0, in1=y0, op=OP.mult)
                nc.vector.tensor_tensor(out=t[:], in0=t[:], in1=ssq_ap, op=OP.mult)
                nc.vector.tensor_scalar(
                    out=t[:], in0=t[:], scalar1=-0.5, scalar2=1.5,
                    op0=OP.mult, op1=OP.add,
                )
                nc.vector.tensor_tensor(out=y1[:], in0=y0, in1=t[:], op=OP.mult)
                nc.vector.tensor_tensor(out=t[:], in0=y1[:], in1=y1[:], op=OP.mult)
                nc.vector.tensor_tensor(out=t[:], in0=t[:], in1=ssq_ap, op=OP.mult)
                nc.vector.tensor_scalar(
                    out=t[:], in0=t[:], scalar1=-0.5, scalar2=1.5,
                    op0=OP.mult, op1=OP.add,
                )
                nc.vector.tensor_tensor(out=inv_ap, in0=y1[:], in1=t[:], op=OP.mult)

            # ---- persistent SBUF state ------------------------------------
            wT = persist.tile([P, scol], bf16, name="wT")
            fa = persist.tile([P, nb * P], f32, name="fa")
            fcast = persist.tile([P, nb * P], bf16, name="fcast")
            fT = persist.tile([P, nb * P], bf16, name="fT")
            ssqF = persist.tile([P, nb], f32, name="ssqF")
            invF = persist.tile([P, nb], f32, name="invF")
            SinvF = persist.tile([P, nb], f32, name="SinvF")
            KSinvF = persist.tile([P, nb], f32, name="KSinvF")
            ssqW = persist.tile([P, nwt], f32, name="ssqW")
            invW = persist.tile([P, nwt], f32, name="invW")
            ssqL = persist.tile([P, nb], f32, name="ssqL")
            invL = persist.tile([P, nb], f32, name="invL")
            ZP = persist.tile([P, nb * nchunk], f32, name="ZP")
            dots = persist.tile([P, nb], f32, name="dots")
            labi = persist.tile([P, nb], i32, name="labi")
            labf = persist.tile([P, nb], f32, name="labf")
            locc = persist.tile([P, nb], f32, name="locc")
            mask = persist.tile([P, nb], f32, name="mask")
            loci = persist.tile([P, nb], i32, name="loci")
            coff = persist.tile([P, 1], f32, name="coff")
            s32row = persist.tile([1, P], f32, name="s32row")
            s32rb = persist.tile([1, P], bf16, name="s32rb")
            s32col = persist.tile([P, 1], bf16, name="s32col")
            wlab = persist.tile([P, nb * D], f32, name="wlab")
            dotsr = persist.tile([P, nb], f32, name="dotsr")
            ccZTL = persist.tile([P, 3 * nb], f32, name="ccZTL")
            RZTL = persist.tile([P, 3 * nb], f32, name="RZTL")
            wbf_d = dramp.tile([scol, D], bf16, name="wbf_d")
            if scol > cs:
                zpad = consts.tile([P, D], bf16, name="zpad")
                nc.vector.memset(zpad[:], 0.0)
                nc.gpsimd.dma_start(
                    out=wbf_d[cs:scol, :], in_=zpad[0 : scol - cs, :]
                )

            # ---- input DMAs ----------------------------------------------
            def issue_wbatch_dma(k):
                r0 = k * wbatch * P
                rows = min(wbatch * P, cs - r0)
                full_t = rows // P
                rem = rows - full_t * P
                nt = full_t + (1 if rem else 0)
                wa = wap.tile([P, wbatch * P], f32, tag="wa", name=f"wa{k}")
                if rem:
                    nc.vector.memset(wa[:, full_t * P : (full_t + 1) * P], 1.0)
                if full_t:
                    nc.sync.dma_start(
                        out=wa[:, 0 : full_t * P].rearrange("p (t d) -> p t d", d=D),
                        in_=w_ext[r0 : r0 + full_t * P, :].rearrange(
                            "(t p) d -> p t d", p=P
                        ),
                    )
                if rem:
                    nc.sync.dma_start(
                        out=wa[0:rem, full_t * P : full_t * P + D],
                        in_=w_ext[r0 + full_t * P : r0 + rows, :],
                    )
                return (wa, nt, r0, rows, full_t, rem)

            was = {}
            was[0] = issue_wbatch_dma(0)  # batch 0 first on the sync queue
            was[1] = issue_wbatch_dma(1)
            nc.vector.memset(s32row[:], 0.0)

            # f + small inputs on the scalar queue (parallel with w)
            nc.scalar.dma_start(
                out=fa[:].rearrange("p (b d) -> p b d", d=D),
                in_=f_ext[:, :].rearrange("(p b) d -> p b d", b=nb),
            )
            nc.scalar.dma_start(out=labi[:], in_=lab_ext[:, :])
            nc.scalar.dma_start(out=coff[:], in_=coff_ext[:, :])

            def prep_wbatch(k):
                """normalize+cast -> transpose into wT; s32 partial."""
                wa, nt, r0, rows, full_t, rem = was[k]
                scrb = smallp.tile([P, wbatch * P], f32, tag="scrb", name="scrb")
                nc.gpsimd.tensor_tensor(
                    out=scrb[:, 0 : nt * P], in0=wa[:, 0 : nt * P],
                    in1=wa[:, 0 : nt * P], op=OP.mult,
                )
                nc.vector.tensor_reduce(
                    out=ssqW[:, k * wbatch : k * wbatch + nt],
                    in_=scrb[:, 0 : nt * P].rearrange("p (t d) -> p t d", d=P),
                    axis=mybir.AxisListType.X,
                    op=OP.add,
                )
                rsqrt(ssqW[:, k * wbatch : k * wbatch + nt],
                      invW[:, k * wbatch : k * wbatch + nt], nt)
                # scale+cast in ONE DVE op: broadcast invW per tile block
                wscb = wscp.tile([P, wbatch * P], bf16, tag="wsc", name="wscb")
                nc.vector.scalar_tensor_tensor(
                    out=wscb[:, 0 : nt * P].rearrange("p (t d) -> p t d", d=P),
                    in0=wa[:, 0 : nt * P].rearrange("p (t d) -> p t d", d=P),
                    scalar=1.0,
                    in1=invW[:, k * wbatch : k * wbatch + nt].broadcast_to(
                        (P, nt, P)
                    ),
                    op0=OP.mult, op1=OP.mult,
                )
                if k < PE_TP:
                    for t in range(nt):
                        gi = k * wbatch + t
                        tpp = psmm.tile([P, P], bf16, tag="pm", name="tp_w")
                        nc.tensor.transpose(
                            out=tpp[:], in_=wscb[:, t * P : (t + 1) * P],
                            identity=ident[:],
                        )
                        nc.vector.tensor_copy(
                            out=wT[:, gi * P : (gi + 1) * P], in_=tpp[:]
                        )
                    if rem:
                        # zero pad columns (pad rows carried memset 1.0)
                        nc.vector.memset(
                            wT[:, r0 + rows : r0 + nt * P], 0.0
                        )
                else:
                    if full_t:
                        nc.gpsimd.dma_start(
                            out=wbf_d[r0 : r0 + full_t * P, :].rearrange(
                                "(t p) d -> p t d", p=P
                            ),
                            in_=wscb[:, 0 : full_t * P].rearrange(
                                "p (t d) -> p t d", d=D
                            ),
                        )
                    if rem:
                        nc.gpsimd.dma_start(
                            out=wbf_d[r0 + full_t * P : r0 + rows, :],
                            in_=wscb[0:rem, full_t * P : full_t * P + D],
                        )
                    bend = min(wbatch * P, scol - r0)
                    nc.sync.dma_start_transpose(
                        out=wT[:, r0 : r0 + bend], in_=wbf_d[r0 : r0 + bend, :]
                    )
                # s32 partial via PE: ones^T @ wscb tiles -> [1, D] row,
                # accumulated in psum across tiles, DVE-added into s32row
                s32p = psmm.tile([P, chunk], f32, tag="pm", name="s32p")
                for t in range(nt):
                    kp = P if (t < full_t or not rem) else rem
                    nc.tensor.matmul(
                        out=s32p[0:1, 0:D],
                        lhsT=onescol[0:kp, 0:1],
                        rhs=wscb[0:kp, t * P : t * P + D],
                        start=(t == 0), stop=(t == nt - 1),
                    )
                nc.vector.tensor_tensor(
                    out=s32row[:], in0=s32row[:], in1=s32p[0:1, 0:D], op=OP.add
                )

            prep_wbatch(0)

            # ---- f prep: ssq only; normalization folded into exp scale ----
            scrF = smallp.tile([P, nb * P], f32, tag="scrb", name="scrF")
            nc.gpsimd.tensor_tensor(out=scrF[:], in0=fa[:], in1=fa[:], op=OP.mult)
            nc.vector.tensor_reduce(
                out=ssqF[:, 0:nb],
                in_=scrF[:].rearrange("p (b d) -> p b d", d=P),
                axis=mybir.AxisListType.X,
                op=OP.add,
            )
            rsqrt(ssqF[:, 0:nb], invF[:, 0:nb], nb)
            nc.vector.tensor_scalar(
                out=SinvF[:], in0=invF[:], scalar1=S, scalar2=None, op0=OP.mult
            )
            nc.vector.tensor_scalar(
                out=KSinvF[:], in0=invF[:], scalar1=float(K_SCHR * S),
                scalar2=None, op0=OP.mult,
            )
            nc.vector.tensor_copy(out=fcast[:], in_=fa[:])
            for b in range(nb):
                sl = slice(b * P, (b + 1) * P)
                tp = psmm.tile([P, P], bf16, tag="pm", name="tp_f")
                nc.tensor.transpose(out=tp[:], in_=fcast[:, sl], identity=ident[:])
                nc.vector.tensor_copy(out=fT[:, sl], in_=tp[:])

            # ---- label localization ---------------------------------------
            nc.vector.tensor_copy(out=labf[:], in_=labi[:])
            nc.vector.tensor_scalar(
                out=locc[:], in0=labf[:], scalar1=coff[:, 0:1], scalar2=0.0,
                op0=OP.subtract, op1=OP.max,
            )
            nc.vector.tensor_scalar(
                out=locc[:], in0=locc[:], scalar1=float(cs - 1), scalar2=None,
                op0=OP.min,
            )
            scrm = smallp.tile([P, nb], f32, tag="scrm", name="scrm")
            nc.vector.tensor_scalar(
                out=scrm[:], in0=labf[:], scalar1=coff[:, 0:1], scalar2=None,
                op0=OP.subtract,
            )
            nc.vector.tensor_tensor(
                out=mask[:], in0=scrm[:], in1=locc[:], op=OP.is_equal
            )
            nc.vector.tensor_copy(out=loci[:], in_=locc[:])

            # ---- main loop ------------------------------------------------
            def issue_wlab(b):
                nc.gpsimd.indirect_dma_start(
                    out=wlab[:, b * D : (b + 1) * D],
                    out_offset=None,
                    in_=w_ext[:, :],
                    in_offset=bass.IndirectOffsetOnAxis(ap=loci[:, b : b + 1], axis=0),
                )

            for cc in range(nbatch):
                if cc + 2 < nbatch:
                    was[cc + 2] = issue_wbatch_dma(cc + 2)
                if cc < 4:
                    for b in range(cc * 4, cc * 4 + 4):
                        issue_wlab(b)
                if cc + 1 < nbatch:
                    prep_wbatch(cc + 1)
                if cc == nbatch - 1:
                    # s32 row complete after batch-6 prep: transpose to a
                    # [P,1] bf16 column via a K=1 matmul, then batched T
                    nc.vector.tensor_copy(out=s32rb[:], in_=s32row[:])
                    tsc = psmm.tile([P, chunk], f32, tag="pm", name="tsc")
                    nc.tensor.matmul(
                        out=tsc[0:P, 0:1], lhsT=s32rb[0:1, 0:P],
                        rhs=onescol[0:1, 0:1], start=True, stop=True,
                    )
                    nc.vector.tensor_copy(out=s32col[:], in_=tsc[0:P, 0:1])
                    tsall = psmm.tile([P, chunk], f32, tag="pm", name="tsall")
                    for b in range(nb):
                        nc.tensor.matmul(
                            out=tsall[:, b : b + 1],
                            lhsT=fT[:, b * P : (b + 1) * P],
                            rhs=s32col[:, 0:1],
                            start=True, stop=True,
                        )
                    nc.vector.tensor_tensor(
                        out=ccZTL[:, nb : 2 * nb], in0=tsall[:, 0:nb],
                        in1=SinvF[:], op=OP.mult,
                    )

                for half in range(nhalf):
                    c0 = cc * gcol + half * chunk
                    cw = min(chunk, scol - c0)
                    if cw <= 0:
                        continue
                    dpos = spread_positions(ND_BY_CC[cc])
                    for b in range(nb):
                        lhs = fT[:, b * P : (b + 1) * P]
                        pm = psmm.tile([P, chunk], f32, tag="pm", name="pm")
                        for sgi in range(math.ceil(cw / 512)):
                            n0 = sgi * 512
                            nn = min(512, cw - n0)
                            nc.tensor.matmul(
                                out=pm[:, n0 : n0 + nn],
                                lhsT=lhs,
                                rhs=wT[:, c0 + n0 : c0 + n0 + nn],
                                start=True, stop=True,
                            )
                        ci = cc * nhalf + half
                        zcol = ZP[:, b * nchunk + ci : b * nchunk + ci + 1]
                        if b in dpos:
                            # DVE Schraudolph: convert, bf16 fold, reduce
                            t16 = i16p.tile([P, chunk], i16, tag="t16",
                                            name="t16")
                            nc.vector.tensor_scalar(
                                out=t16[:, 0:cw], in0=pm[:, 0:cw],
                                scalar1=KSinvF[:, b : b + 1], scalar2=bimm,
                                op0=OP.mult, op1=OP.add,
                            )
                            if DVE_FOLD and cw == chunk:
                                h = cw // 2
                                fold = i16p.tile([P, chunk // 2], bf16,
                                                 tag="fold", name="fold")
                                nc.vector.tensor_tensor(
                                    out=fold[:, 0:h],
                                    in0=t16[:, 0:h].bitcast(bf16),
                                    in1=t16[:, h : 2 * h].bitcast(bf16),
                                    op=OP.add,
                                )
                                nc.vector.tensor_reduce(
                                    out=zcol, in_=fold[:, 0:h],
                                    axis=mybir.AxisListType.X, op=OP.add,
                                )
                            else:
                                nc.vector.tensor_reduce(
                                    out=zcol, in_=t16[:, 0:cw].bitcast(bf16),
                                    axis=mybir.AxisListType.X, op=OP.add,
                                )
                        else:
                            nc.scalar.activation(
                                out=pm[:, 0:cw], in_=pm[:, 0:cw], func=FT.Exp,
                                bias=bias_m30[:, 0:1],
                                scale=SinvF[:, b : b + 1],
                                accum_out=zcol,
                            )

            # ---- label-dot columns (batched [P, nb*D] ops) ----------------
            scrw = smallp.tile([P, nb * D], f32, tag="scrb", name="scrw")
            nc.gpsimd.tensor_tensor(
                out=scrw[:], in0=wlab[:], in1=wlab[:], op=OP.mult
            )
            nc.vector.tensor_reduce(
                out=ssqL[:, 0:nb],
                in_=scrw[:].rearrange("p (b d) -> p b d", d=D),
                axis=mybir.AxisListType.X, op=OP.add,
            )
            rsqrt(ssqL[:, 0:nb], invL[:, 0:nb], nb)
            scrw2 = smallp.tile([P, nb * D], f32, tag="scrb", name="scrw2")
            nc.gpsimd.tensor_tensor(
                out=scrw2[:], in0=wlab[:], in1=fa[:], op=OP.mult
            )
            nc.vector.tensor_reduce(
                out=dotsr[:],
                in_=scrw2[:].rearrange("p (b d) -> p b d", d=D),
                axis=mybir.AxisListType.X, op=OP.add,
            )
            nc.vector.tensor_tensor(out=dots[:], in0=dotsr[:], in1=invL[:], op=OP.mult)
            scrl = smallp.tile([P, nb], f32, tag="scrm", name="scrl")
            nc.vector.tensor_tensor(out=scrl[:], in0=dots[:], in1=mask[:], op=OP.mult)
            nc.vector.tensor_tensor(
                out=ccZTL[:, 2 * nb : 3 * nb], in0=scrl[:], in1=SinvF[:], op=OP.mult
            )

            # ---- Z columns ------------------------------------------------
            nc.vector.tensor_reduce(
                out=ccZTL[:, 0:nb],
                in_=ZP[:].rearrange("p (b c) -> p b c", c=nchunk),
                axis=mybir.AxisListType.X,
                op=OP.add,
            )

            # ---- AllReduce(add) of [Z|T|L] --------------------------------
            cz_in = dramp.tile([P, 3 * nb], f32, name="cz_in")
            cz_out = dramp.tile([P, 3 * nb], f32, name="cz_out")
            nc.sync.dma_start(out=cz_in[:], in_=ccZTL[:])
            nc.gpsimd.collective_compute(
                "AllReduce",
                OP.add,
                replica_groups=[list(range(num_cores))],
                ins=[cz_in.opt()],
                outs=[cz_out.opt()],
            )
            nc.sync.dma_start(out=RZTL[:], in_=cz_out[:])

            # ---- final loss -----------------------------------------------
            Zg = RZTL[:, 0:nb]
            Tg = RZTL[:, nb : 2 * nb]
            Lg = RZTL[:, 2 * nb : 3 * nb]
            expL = smallp.tile([P, nb], f32, tag="fin", name="expL")
            zadj = smallp.tile([P, nb], f32, tag="fin2", name="zadj")
            lnz = smallp.tile([P, nb], f32, tag="fin3", name="lnz")
            t1 = smallp.tile([P, nb], f32, tag="fin4", name="t1")
            losscol = smallp.tile([P, 1], f32, tag="fin6", name="losscol")
            outsb = smallp.tile([1, 1], f32, tag="fin8", name="outsb")

            nc.scalar.activation(
                out=expL[:], in_=Lg, func=FT.Exp, bias=bias_m30[:, 0:1], scale=1.0
            )
            nc.vector.scalar_tensor_tensor(
                out=zadj[:], in0=expL[:], scalar=kappa_m1, in1=Zg,
                op0=OP.mult, op1=OP.add,
            )
            # DVE bitcast-ln: ln(x) ~= (bits*2^-23 - 126.94269504)*ln2
            nc.vector.tensor_copy(out=lnz[:], in_=zadj[:].bitcast(i32))
            nc.vector.tensor_scalar(
                out=lnz[:], in0=lnz[:], scalar1=float(LN2 / (1 << 23)),
                scalar2=float(-126.94269504 * LN2), op0=OP.mult, op1=OP.add,
            )
            nc.vector.scalar_tensor_tensor(
                out=t1[:], in0=Lg, scalar=-(1.0 - EPS), in1=lnz[:],
                op0=OP.mult, op1=OP.add,
            )
            nc.vector.scalar_tensor_tensor(
                out=t1[:], in0=Tg, scalar=-EPS / c_total, in1=t1[:],
                op0=OP.mult, op1=OP.add,
            )
            nc.vector.tensor_reduce(
                out=losscol[:], in_=t1[:], axis=mybir.AxisListType.X, op=OP.add
            )
            ones_ap = nc.const_aps.aps[(f32, 1.0)]
            pfin = psmm.tile([P, 16], f32, tag="pm", name="pfin")
            nc.tensor.matmul(
                out=pfin[0:1, 0:1], lhsT=losscol[:, 0:1], rhs=ones_ap[:, 0:1],
                start=True, stop=True,
            )
            nc.vector.tensor_scalar(
                out=outsb[:], in0=pfin[0:1, 0:1], scalar1=1.0 / b_sz,
                scalar2=const, op0=OP.mult, op1=OP.add,
            )
            nc.sync.dma_start(out=out_ext[:, :], in_=outsb[:])

    nc.compile()
    return nc


def make_in_maps(features, labels, weight, num_cores=NCORES, b_sz=B, cs=CS):
    nb = b_sz // P
    f = np.ascontiguousarray(np.asarray(features, dtype=np.float32))
    lab = np.ascontiguousarray(np.asarray(labels).astype(np.int32).reshape(P, nb))
    w = np.asarray(weight, dtype=np.float32)
    in_maps = []
    for k in range(num_cores):
        in_maps.append(
            {
                "f": f,
                "w": np.ascontiguousarray(w[k * cs : (k + 1) * cs]),
                "lab": lab,
                "coff": np.full((P, 1), k * cs, dtype=np.float32),
                "id32": np.eye(P, dtype=np.float32),
            }
        )
    return in_maps


_NC_CACHE = {}


def kernel(features, labels, weight):
    from concourse.bass_utils import run_bass_kernel_spmd

    if "nc" not in _NC_CACHE:
        _NC_CACHE["nc"] = build_graph()
    nc = _NC_CACHE["nc"]
    in_maps = make_in_maps(features, labels, weight)
    res = run_bass_kernel_spmd(nc, in_maps, core_ids=list(range(NCORES)))
    return np.float32(res.results[0]["out"][0, 0])
